# revision 36
# baseline (speedup 1.0000x reference)
"""GQA attention (int8-quantized QK^T, RoPE, causal softmax) on 8 TRN2 NeuronCores.

Sharding: tensor-parallel over heads. Core c owns Q heads 4c..4c+3 (Wq cols
512c..512c+512), KV head c (Wk/Wv cols 128c..128c+128), and Wo rows
512c..512c+512. x is replicated. Each core emits a partial [2048, 4096]
bf16 output (its heads' contribution through Wo); the host sums the 8
partials in float64. No on-device collectives.

Per-core dataflow (matmuls in bf16; QK^T is exact: int-quantized values are
integers <= 127, exactly representable in bf16, accumulated in fp32 PSUM):
  A) x -> bf16 DRAM scratch (column-chunked casts) -> x^T via hardware
     DMA-transpose on the Sync queue; Q/K/V projections in natural [s, f]
     layout; RoPE + absmax-quantize on VectorE; PE-transpose q/k to [hd, s].
  B) scores^T [t, q] = kT-slice.T @ qT-block; dequant via
     scalar_tensor_tensor (k-scale per-partition, q-scale broadcast from
     gpsimd partition_broadcast); exp on ScalarE; causal zeroing of
     diagonal-band tiles on gpsimd post-exp; den = ones.T @ P^T;
     O^T += V-chunk.T @ P^T; heads processed in pairs so TensorE always has
     the sibling head's matmuls while one head's dequant/exp round-trips.
  C) out[s, :] += OT-slice.T @ Wo-chunk accumulated over f, DMA out (bf16)
     on the Scalar queue.
"""

import numpy as np

import concourse.bass as bass
import concourse.mybir as mybir
import concourse.tile as tile
from concourse import bacc
from concourse.bass_utils import run_bass_kernel_spmd
from concourse.masks import make_identity

FP = mybir.dt.float32
BF = mybir.dt.bfloat16
AL = mybir.AluOpType
AF = mybir.ActivationFunctionType

B, S, D, NH, NKV, HD = 1, 2048, 4096, 32, 8, 128
NCORES = 8
HPC = NH // NCORES          # 4 Q heads per core
FQ = HPC * HD               # 512
SCALE = HD ** -0.5
MAGIC = 3 * 2.0 ** 22       # fp32 round-to-nearest-even magic constant

ST = S // 128               # 16 s-tiles of 128 rows
DC = D // 128               # 32 d-chunks
NJ = S // 512               # 4 q-blocks of 512
SBLK = 4                    # s-tiles per x-transpose block (512 rows)
NB = ST // SBLK


def build_graph():
    nc = bacc.Bacc(None)
    x_e = nc.declare_dram_parameter("x", [S, D], FP, isOutput=False)
    wq_e = nc.declare_dram_parameter("wq", [D, FQ], FP, isOutput=False)
    wk_e = nc.declare_dram_parameter("wk", [D, HD], FP, isOutput=False)
    wv_e = nc.declare_dram_parameter("wv", [D, HD], FP, isOutput=False)
    wo_e = nc.declare_dram_parameter("wo", [FQ, D], FP, isOutput=False)
    cos_e = nc.declare_dram_parameter("cos", [S, HD], FP, isOutput=False)
    sin_e = nc.declare_dram_parameter("sin", [S, HD], FP, isOutput=False)
    out_e = nc.declare_dram_parameter("out", [S, D], BF, isOutput=True)

    with tile.TileContext(nc) as tc:
        with (
            tc.tile_pool(name="persist", bufs=1) as pp,
        ):
            ident = pp.tile([128, 128], BF)
            make_identity(nc, ident[:])
            ones1 = pp.tile([128, 1], BF)       # den stationary (M=1)
            nc.gpsimd.memset(ones1[:], 1.0)

            qT = pp.tile([128, HPC, S], BF)     # quantized Q^T per head
            kT = pp.tile([128, S], BF)          # quantized K^T
            vn = pp.tile([128, ST, HD], BF)     # V natural, per t-chunk
            qsrT = pp.tile([1, HPC, S], BF)     # q dequant scale rows (partition 0)
            ksr = pp.tile([128, ST], FP)        # k dequant scale (SCALE folded)
            OT = pp.tile([128, HPC, S], BF)     # normalized O^T per head

            # ---------------- Phase A: x^T, projections, RoPE, quantize
            with (
                tc.tile_pool(name="ropec", bufs=1) as rp,
                tc.tile_pool(name="xtp", bufs=2) as xtp,
                tc.tile_pool(name="dram", bufs=1, space="DRAM") as drp,
                tc.tile_pool(name="wq", bufs=1) as wqp,
                tc.tile_pool(name="ab", bufs=2) as ab,
                tc.tile_pool(name="psA", bufs=2, space="PSUM") as psA,
                tc.tile_pool(name="psA1", bufs=2, space="PSUM") as psA1,
            ):
                # SWDGE queue order == emission order. x casts are
                # column-chunked so each block's transposes can begin after
                # one quarter of its cast lands.
                xdrs = []
                for blk in range(NB):
                    xdr = drp.tile([SBLK * 128, D], BF, tag=f"xdr{blk}")
                    xdrs.append(xdr)
                nc.gpsimd.dma_start(xdrs[0][:], x_e[0:SBLK * 128, :])
                wqr = wqp.tile([128, DC, FQ], BF)
                for wc in range(4):
                    nc.gpsimd.dma_start(
                        wqr[:, wc * 8:(wc + 1) * 8, :],
                        wq_e[:].rearrange("(c p) f -> p c f", p=128)[:, wc * 8:(wc + 1) * 8, :])
                wkv = rp.tile([128, DC, 2 * HD], BF)
                nc.gpsimd.dma_start(wkv[:, :, 0:HD], wk_e[:].rearrange("(c p) h -> p c h", p=128))
                nc.gpsimd.dma_start(wkv[:, :, HD:2 * HD], wv_e[:].rearrange("(c p) h -> p c h", p=128))
                for blk in range(1, NB):
                    r0 = blk * SBLK * 128
                    nc.gpsimd.dma_start(xdrs[blk][:], x_e[r0:r0 + SBLK * 128, :])

                cosr = rp.tile([128, ST, HD], FP)
                sinm = rp.tile([128, ST, HD], FP)   # [-sin | +sin] halves
                nc.sync.dma_start(cosr[:], cos_e[:].rearrange("(t p) d -> p t d", p=128))
                nc.sync.dma_start(sinm[:], sin_e[:].rearrange("(t p) d -> p t d", p=128))
                nc.vector.tensor_scalar_mul(sinm[:, :, 0:64], sinm[:, :, 0:64], -1.0)
                idf = rp.tile([128, 128], FP)
                make_identity(nc, idf[:])

                for blk in range(NB):
                    xTs = []
                    for d in range(DC):
                        xTd = xtp.tile([128, SBLK * 128], BF, tag=f"xT{d}")
                        xTs.append(xTd)
                    for d in range(DC):
                        nc.sync.dma_start(
                            xTs[d][:],
                            xdrs[blk][:, d * 128:(d + 1) * 128],
                            transpose=True)

                    for i in range(SBLK):
                        st_i = blk * SBLK + i
                        q_ps = psA.tile([128, FQ], FP, tag="qps")
                        kv_ps = psA.tile([128, 2 * HD], FP, tag="kvps")
                        for d in range(DC):
                            nc.tensor.matmul(q_ps[:], xTs[d][:, i * 128:(i + 1) * 128],
                                             wqr[:, d, :],
                                             start=(d == 0), stop=(d == DC - 1))
                        for d in range(DC):
                            nc.tensor.matmul(kv_ps[:], xTs[d][:, i * 128:(i + 1) * 128],
                                             wkv[:, d, :],
                                             start=(d == 0), stop=(d == DC - 1))

                        # V natural: straight cast
                        nc.scalar.copy(vn[:, st_i, :], kv_ps[:, HD:2 * HD])

                        # RoPE + quantize q (4 heads) and k (1 head)
                        qi = ab.tile([128, FQ], BF, tag="qi")
                        ki = ab.tile([128, HD], BF, tag="ki")
                        for (src, nh, i8out) in ((q_ps, HPC, qi), (kv_ps, 1, ki)):
                            rr = ab.tile([128, nh, HD], FP, tag=f"rr{nh}")
                            t2 = ab.tile([128, nh, HD], FP, tag=f"t2{nh}")
                            am = ab.tile([128, nh], FP, tag=f"am{nh}")
                            sc = ab.tile([128, nh], FP, tag=f"sc{nh}")
                            for h in range(nh):
                                co = cosr[:, st_i, :]
                                si = sinm[:, st_i, :]
                                nc.vector.tensor_mul(rr[:, h, :], src[:, h * HD:(h + 1) * HD], co)
                                nc.vector.tensor_mul(t2[:, h, 0:64], src[:, h * HD + 64:(h + 1) * HD], si[:, 0:64])
                                nc.vector.tensor_mul(t2[:, h, 64:HD], src[:, h * HD:h * HD + 64], si[:, 64:HD])
                            nc.vector.tensor_add(rr[:], rr[:], t2[:])
                            nc.vector.tensor_reduce(am[:], rr[:], axis=mybir.AxisListType.X,
                                                    op=AL.max, apply_absolute_value=True)
                            nc.vector.tensor_scalar_max(am[:], am[:], 1e-5)
                            nc.vector.reciprocal_approx_fast(sc[:], am[:])   # ~1/amax
                            for h in range(nh):
                                nc.vector.tensor_scalar(rr[:, h, :], rr[:, h, :],
                                                        sc[:, h:h + 1], None, op0=AL.mult)
                            nc.vector.tensor_scalar(rr[:], rr[:], 127.0, MAGIC, op0=AL.mult, op1=AL.add)
                            nc.vector.tensor_scalar(i8out[:], rr[:], MAGIC, None, op0=AL.subtract)
                            if nh == 1:
                                nc.vector.tensor_scalar_mul(ksr[:, st_i:st_i + 1], am[:], SCALE / 127.0)
                            else:
                                nc.vector.tensor_scalar_mul(am[:], am[:], 1.0 / 127.0)
                                for h in range(HPC):
                                    qsr_ps = psA1.tile([1, 128], FP, tag="qsrtp")
                                    nc.tensor.transpose(qsr_ps[:], am[:, h:h + 1], idf[:])
                                    nc.scalar.copy(qsrT[0:1, h, st_i * 128:(st_i + 1) * 128],
                                                   qsr_ps[:])

                        # transpose quantized q/k into [hd, s] layout via PE
                        for h in range(HPC):
                            tp = psA.tile([128, 128], BF, tag="tp")
                            nc.tensor.transpose(tp[:], qi[:, h * HD:(h + 1) * HD], ident[:])
                            nc.scalar.copy(qT[:, h, st_i * 128:(st_i + 1) * 128], tp[:])
                        tp = psA.tile([128, 128], BF, tag="tp")
                        nc.tensor.transpose(tp[:], ki[:], ident[:])
                        nc.scalar.copy(kT[:, st_i * 128:(st_i + 1) * 128], tp[:])

            # ---------------- Phase B: attention (Wo prefetched meanwhile)
            wop_cm = tc.tile_pool(name="wo", bufs=1)
            wop = wop_cm.__enter__()
            wo_r = wop.tile([128, HPC, D], BF)
            nc.gpsimd.dma_start(wo_r[:], wo_e[:].rearrange("(f p) d -> p f d", p=128))
            with (
                tc.tile_pool(name="att", bufs=3) as at,
                tc.tile_pool(name="attf", bufs=4) as atf,
                tc.tile_pool(name="psSC", bufs=4, space="PSUM") as psSC,
                tc.tile_pool(name="psO", bufs=2, space="PSUM") as psO,
                tc.tile_pool(name="psDen", bufs=1, space="PSUM") as psDen,
            ):
                for J in range(NJ):
                    nlive = 4 * J + 4
                    for h in range(HPC):
                        dqb = at.tile([128, 512], BF, tag="dqb")
                        nc.gpsimd.partition_broadcast(
                            dqb[:], qsrT[0:1, h, J * 512:(J + 1) * 512])
                        oT_ps = psO.tile([128, 512], FP, tag="o")
                        den_ps = psDen.tile([1, 512], FP, tag="den")
                        for ti in range(nlive):
                            sc_ps = psSC.tile([128, 512], FP, tag="sc")
                            nc.tensor.matmul(sc_ps[:], kT[:, ti * 128:(ti + 1) * 128],
                                             qT[:, h, J * 512:(J + 1) * 512])
                            ptf = atf.tile([128, 512], FP, tag="ptf")
                            nc.vector.scalar_tensor_tensor(
                                out=ptf[:], in0=sc_ps[:], scalar=ksr[:, ti:ti + 1],
                                in1=dqb[:], op0=AL.mult, op1=AL.mult)
                            pt = atf.tile([128, 512], BF, tag="pt")
                            nc.scalar.activation(pt[:], ptf[:], AF.Exp)
                            if ti >= 4 * J:
                                nc.gpsimd.affine_select(
                                    out=pt[:], in_=pt[:],
                                    compare_op=AL.is_ge, fill=0.0,
                                    base=J * 512 - ti * 128, channel_multiplier=-1,
                                    pattern=[[1, 512]])
                            nc.tensor.matmul(den_ps[:], ones1[:], pt[:],
                                             start=(ti == 0), stop=(ti == nlive - 1))
                            nc.tensor.matmul(oT_ps[:], vn[:, ti, :], pt[:],
                                             start=(ti == 0), stop=(ti == nlive - 1))
                        denr = at.tile([1, 512], FP, tag="denr")
                        nc.vector.reciprocal_approx_fast(denr[:], den_ps[:])
                        dnb = at.tile([128, 512], FP, tag="dnb")
                        nc.gpsimd.partition_broadcast(dnb[:], denr[:])
                        nc.vector.tensor_mul(OT[:, h, J * 512:(J + 1) * 512],
                                             oT_ps[:], dnb[:])

            # ---------------- Phase C: output projection (partial sums)
            with (
                tc.tile_pool(name="ost", bufs=2) as ost,
                tc.tile_pool(name="psC", bufs=4, space="PSUM") as psC,
            ):
                for st_i in range(ST):
                    for half in range(2):
                        ot_sb = ost.tile([128, D // 2], BF, tag="ot")
                        for dbl in range(4):
                            db = half * 4 + dbl
                            wo_ps = psC.tile([128, 512], FP, tag="wo")
                            for f in range(HPC):
                                nc.tensor.matmul(wo_ps[:], OT[:, f, st_i * 128:(st_i + 1) * 128],
                                                 wo_r[:, f, db * 512:(db + 1) * 512],
                                                 start=(f == 0), stop=(f == HPC - 1))
                            if db % 2 == 0:
                                nc.scalar.copy(ot_sb[:, dbl * 512:(dbl + 1) * 512], wo_ps[:])
                            else:
                                nc.vector.tensor_copy(ot_sb[:, dbl * 512:(dbl + 1) * 512], wo_ps[:])
                        nc.scalar.dma_start(
                            out_e[st_i * 128:(st_i + 1) * 128,
                                  half * (D // 2):(half + 1) * (D // 2)],
                            ot_sb[:])
            wop_cm.__exit__(None, None, None)

    nc.compile()
    return nc


_CACHE = {}


def kernel(x, Wq, Wk, Wv, Wo, cos, sin):
    x2 = np.ascontiguousarray(np.asarray(x, np.float32).reshape(S, D))
    in_maps = []
    for c in range(NCORES):
        in_maps.append({
            "x": x2,
            "wq": np.ascontiguousarray(Wq[:, c * FQ:(c + 1) * FQ], np.float32),
            "wk": np.ascontiguousarray(Wk[:, c * HD:(c + 1) * HD], np.float32),
            "wv": np.ascontiguousarray(Wv[:, c * HD:(c + 1) * HD], np.float32),
            "wo": np.ascontiguousarray(Wo[c * FQ:(c + 1) * FQ, :], np.float32),
            "cos": np.ascontiguousarray(cos, np.float32),
            "sin": np.ascontiguousarray(sin, np.float32),
        })
    if "nc" not in _CACHE:
        _CACHE["nc"] = build_graph()
    res = run_bass_kernel_spmd(_CACHE["nc"], in_maps, core_ids=list(range(NCORES)))
    out = np.zeros((S, D), np.float64)
    for r in res.results:
        out += np.asarray(r["out"], np.float64)
    return out.astype(np.float32).reshape(B, S, D)
